# revision 3
# baseline (speedup 1.0000x reference)
"""Trainium2 Bass kernel for FFT-conv1d (= valid cross-correlation conv1d).

Reference computes, for x[N=64, C=64, W=4096], w[F=64, C=64, WW=16], b[F=64]:
    out[n, f, t] = sum_{c, j} x[n, c, t + j] * w[f, c, j] + b[f],  t in [0, 4081)

Strategy (v3, per-output-tile x windows + bf16 stores):
  - Data-parallel: shard N across 8 NeuronCores (8 samples per core);
    replicate w and b.
  - Direct convolution on the TensorEngine in bf16 (inputs cast on host;
    fp32 PSUM accumulation):
      * K = 128 contraction: channels c (64) x 2 adjacent taps.  Partitions
        0-63 hold x[n,c,t], partitions 64-127 hold x[n,c,t+1]; both halves
        are loaded straight from HBM (the shifted half re-reads x at +1) on
        two independent HWDGE queues, so there is no load->shift chain.
      * Each x tile covers exactly one 512-wide output tile (528 cols incl.
        the 14-tap halo), so the first matmul only waits for ~270 KB of DMA
        instead of a whole sample pair.
      * M = 64 output channels; the two samples of a pair go to PE column
        groups 0 / 64 (PSUM partitions 0-63 / 64-127), issued interleaved
        (jb outer, s inner) so both streams overlap on the 128x128 array.
      * 8 matmuls (tap pairs) accumulate one PSUM bank [128, 512].
  - Evacuation: one DVE tensor_scalar_add per bank (PSUM -> SBUF bf16,
    fused per-channel bias); bf16 stores halve the output DMA traffic.
    Host casts the bf16 result back to fp32.
"""

import numpy as np

N, C, W = 64, 64, 4096
F, WW = 64, 16
OUT_W = W - WW + 1  # 4081
N_CORES = 8
NPC = N // N_CORES  # samples per core = 8
NKT = 8  # output tiles of 512 per sample
TW = 528  # x tile width: 2*7 + 512 = 526 cols used, padded to 528

_CACHE = {}


def _build_nc():
    from contextlib import ExitStack

    import concourse.bacc as bacc
    import concourse.mybir as mybir
    import concourse.tile as tile

    f32 = mybir.dt.float32
    bf16 = mybir.dt.bfloat16

    nc = bacc.Bacc(
        "TRN2", target_bir_lowering=False, debug=False, num_devices=N_CORES
    )
    x_d = nc.dram_tensor("x", [NPC, C, W], bf16, kind="ExternalInput").ap()
    w_d = nc.dram_tensor("wstk", [128, 512], bf16, kind="ExternalInput").ap()
    b_d = nc.dram_tensor("bias2", [128, 1], f32, kind="ExternalInput").ap()
    o_d = nc.dram_tensor("out", [NPC, F, OUT_W], bf16, kind="ExternalOutput").ap()

    with tile.TileContext(nc) as tc:
        with ExitStack() as ctx:
            consts = ctx.enter_context(tc.tile_pool(name="consts", bufs=1))
            xpool = ctx.enter_context(tc.tile_pool(name="xs", bufs=32))
            opool = ctx.enter_context(tc.tile_pool(name="osb", bufs=6))
            pspool = ctx.enter_context(
                tc.tile_pool(name="ps", bufs=8, space="PSUM")
            )

            # w/bias ride the gpsimd queue so the first sync/scalar-queue
            # DMAs are x data and compute starts as early as possible
            wsb = consts.tile([128, 512], bf16)
            nc.gpsimd.dma_start(out=wsb[:, :], in_=w_d[:, :])
            bsb = consts.tile([128, 1], f32)
            nc.gpsimd.dma_start(out=bsb[:, :], in_=b_d[:, :])

            for pair in range(NPC // 2):
                xts = []
                for kt in range(NKT):
                    base = kt * 512
                    w0 = min(TW, W - base)  # straight cols available
                    w1 = min(TW, W - base - 1)  # shifted cols available
                    pkt = []
                    for s in range(2):
                        n = 2 * pair + s
                        xt = xpool.tile([128, TW], bf16)
                        if w0 < TW:
                            nc.vector.memset(xt[0:64, w0:TW], 0.0)
                        if w1 < TW:
                            nc.vector.memset(xt[64:128, w1:TW], 0.0)
                        nc.sync.dma_start(
                            out=xt[0:64, 0:w0], in_=x_d[n, :, base : base + w0]
                        )
                        nc.scalar.dma_start(
                            out=xt[64:128, 0:w1],
                            in_=x_d[n, :, base + 1 : base + 1 + w1],
                        )
                        pkt.append(xt)
                    xts.append(pkt)

                for kt in range(NKT):
                    ps = pspool.tile([128, 512], f32)
                    # sample s -> PE column group 64*s; jb outer / s inner
                    # so the two column-group streams interleave and run
                    # concurrently on disjoint array column halves.
                    for jb in range(8):
                        for s in range(2):
                            nc.tensor.matmul(
                                ps[64 * s : 64 * (s + 1), :],
                                lhsT=wsb[:, jb * 64 : (jb + 1) * 64],
                                rhs=xts[kt][s][:, 2 * jb : 2 * jb + 512],
                                start=(jb == 0),
                                stop=(jb == 7),
                            )
                    osb = opool.tile([128, 512], bf16)
                    nc.vector.tensor_scalar_add(
                        osb[:, :], ps[:, :], bsb[:, 0:1]
                    )
                    lo = kt * 512
                    wcols = min(512, OUT_W - lo)
                    eng = nc.sync if (pair == NPC // 2 - 1 and kt == 7) else nc.gpsimd
                    eng.dma_start(
                        out=o_d[2 * pair : 2 * pair + 2].flatten_outer_dims()[
                            :, lo : lo + wcols
                        ],
                        in_=osb[:, 0:wcols],
                    )

    nc.compile()
    return nc


def _get_nc():
    if "nc" not in _CACHE:
        _CACHE["nc"] = _build_nc()
    return _CACHE["nc"]


def _host_prep(w, b):
    import ml_dtypes

    # wstk[p*64 + c, jb*64 + f] = w[f, c, 2*jb + p]
    arr = np.ascontiguousarray(w, dtype=np.float32).reshape(F, C, 8, 2)
    wstk = np.ascontiguousarray(
        arr.transpose(3, 1, 2, 0).reshape(128, 512).astype(ml_dtypes.bfloat16)
    )
    bias2 = np.concatenate([b, b]).astype(np.float32).reshape(128, 1)
    bias2 = np.ascontiguousarray(bias2)
    return wstk, bias2


def kernel(x, w, b):
    import ml_dtypes

    from concourse.bass_utils import run_bass_kernel_spmd

    x = np.asarray(x, dtype=np.float32)
    w = np.asarray(w, dtype=np.float32)
    b = np.asarray(b, dtype=np.float32)
    assert x.shape == (N, C, W) and w.shape == (F, C, WW) and b.shape == (F,)

    nc = _get_nc()
    wstk, bias2 = _host_prep(w, b)
    xbf = np.ascontiguousarray(x.astype(ml_dtypes.bfloat16))
    in_maps = [
        {
            "x": np.ascontiguousarray(xbf[i * NPC : (i + 1) * NPC]),
            "wstk": wstk,
            "bias2": bias2,
        }
        for i in range(N_CORES)
    ]
    res = run_bass_kernel_spmd(nc, in_maps, core_ids=list(range(N_CORES)))
    out = np.concatenate([r["out"] for r in res.results], axis=0)
    return out.astype(np.float32)


# revision 4
# speedup vs baseline: 1.1274x; 1.1274x over previous
"""Trainium2 Bass kernel for FFT-conv1d (= valid cross-correlation conv1d).

Reference computes, for x[N=64, C=64, W=4096], w[F=64, C=64, WW=16], b[F=64]:
    out[n, f, t] = sum_{c, j} x[n, c, t + j] * w[f, c, j] + b[f],  t in [0, 4081)

Strategy (v4, dual HBM loads + bf16 stores):
  - Data-parallel: shard N across 8 NeuronCores (8 samples per core);
    replicate w and b.
  - Direct convolution on the TensorEngine in bf16 (inputs cast on host;
    fp32 PSUM accumulation):
      * K = 128 contraction: channels c (64) x 2 adjacent taps.  Partitions
        0-63 hold x[n,c,t], partitions 64-127 hold x[n,c,t+1].  BOTH halves
        are loaded straight from HBM (the shifted half re-reads x at +1) as
        one large per-sample DMA each, on two independent HWDGE queues -
        few large DMAs (HWDGE issue cost is ~1us per dma_start), no
        load->shift dependency chain.
      * M = 64 output channels; the two samples of a pair go to PE column
        groups 0 / 64 (PSUM partitions 0-63 / 64-127), issued interleaved
        (jb outer, s inner) so both streams overlap on the 128x128 array.
      * 8 matmuls (tap pairs) accumulate one PSUM bank [128, 512]
        = 512 output positions for two samples.
  - Evacuation: one DVE tensor_scalar_add per bank (PSUM -> SBUF bf16,
    fused per-channel bias).  bf16 stores halve output DMA traffic; the
    host casts back to fp32.  Output slices ship on the lightly-loaded
    HWDGE queues while later tiles compute.
"""

import numpy as np

N, C, W = 64, 64, 4096
F, WW = 64, 16
OUT_W = W - WW + 1  # 4081
N_CORES = 8
NPC = N // N_CORES  # samples per core = 8
XPAD = 4112  # padded xs width (max col read = 7*512 + 14 + 511 = 4109)
NKT = 8  # output tiles of 512 per sample

_CACHE = {}


def _build_nc():
    from contextlib import ExitStack

    import concourse.bacc as bacc
    import concourse.mybir as mybir
    import concourse.tile as tile

    f32 = mybir.dt.float32
    bf16 = mybir.dt.bfloat16

    nc = bacc.Bacc(
        "TRN2", target_bir_lowering=False, debug=False, num_devices=N_CORES
    )
    x_d = nc.dram_tensor("x", [NPC, C, W], bf16, kind="ExternalInput").ap()
    w_d = nc.dram_tensor("wstk", [128, 512], bf16, kind="ExternalInput").ap()
    b_d = nc.dram_tensor("bias2", [128, 1], f32, kind="ExternalInput").ap()
    o_d = nc.dram_tensor("out", [NPC, F, OUT_W], bf16, kind="ExternalOutput").ap()

    with tile.TileContext(nc) as tc:
        with ExitStack() as ctx:
            consts = ctx.enter_context(tc.tile_pool(name="consts", bufs=1))
            xpool = ctx.enter_context(tc.tile_pool(name="xs", bufs=6))
            opool = ctx.enter_context(tc.tile_pool(name="osb", bufs=3))
            pspool = ctx.enter_context(
                tc.tile_pool(name="ps", bufs=8, space="PSUM")
            )

            # w/bias ride the gpsimd queue so the first sync/scalar-queue
            # DMAs are x data and compute starts as early as possible
            wsb = consts.tile([128, 512], bf16)
            nc.gpsimd.dma_start(out=wsb[:, :], in_=w_d[:, :])
            bsb = consts.tile([128, 1], f32)
            nc.gpsimd.dma_start(out=bsb[:, :], in_=b_d[:, :])

            for pair in range(NPC // 2):
                xs = []
                for s in range(2):
                    n = 2 * pair + s
                    xt = xpool.tile([128, XPAD], bf16)
                    nc.vector.memset(xt[0:64, W:XPAD], 0.0)
                    nc.vector.memset(xt[64:128, W - 1 : XPAD], 0.0)
                    # straight half on the sync HWDGE queue, shifted half
                    # on the scalar HWDGE queue: one big DMA each, running
                    # on complementary SDMA engine sets.
                    nc.sync.dma_start(out=xt[0:64, 0:W], in_=x_d[n, :, :])
                    nc.scalar.dma_start(
                        out=xt[64:128, 0 : W - 1], in_=x_d[n, :, 1:W]
                    )
                    xs.append(xt)

                osb = opool.tile([128, NKT * 512], bf16)
                for kt in range(NKT):
                    ps = pspool.tile([128, 512], f32)
                    # sample s -> PE column group 64*s; jb outer / s inner
                    # so the two column-group streams interleave and run
                    # concurrently on disjoint array column halves.
                    for jb in range(8):
                        for s in range(2):
                            nc.tensor.matmul(
                                ps[64 * s : 64 * (s + 1), :],
                                lhsT=wsb[:, jb * 64 : (jb + 1) * 64],
                                rhs=xs[s][
                                    :, kt * 512 + 2 * jb : kt * 512 + 2 * jb + 512
                                ],
                                start=(jb == 0),
                                stop=(jb == 7),
                            )
                    nc.vector.tensor_scalar_add(
                        osb[:, kt * 512 : (kt + 1) * 512], ps[:, :], bsb[:, 0:1]
                    )
                    # ship finished output slices while later tiles compute;
                    # stores go on the lightly-loaded HWDGE queues
                    ocuts = {
                        3: (0, 2048, nc.scalar),
                        5: (2048, 3072, nc.sync),
                        6: (3072, 3584, nc.scalar),
                        7: (3584, OUT_W, nc.sync),
                    }
                    if kt in ocuts:
                        lo, hi, eng = ocuts[kt]
                        eng.dma_start(
                            out=o_d[2 * pair : 2 * pair + 2].flatten_outer_dims()[
                                :, lo:hi
                            ],
                            in_=osb[:, lo:hi],
                        )

    nc.compile()
    return nc


def _get_nc():
    if "nc" not in _CACHE:
        _CACHE["nc"] = _build_nc()
    return _CACHE["nc"]


def _host_prep(w, b):
    import ml_dtypes

    # wstk[p*64 + c, jb*64 + f] = w[f, c, 2*jb + p]
    arr = np.ascontiguousarray(w, dtype=np.float32).reshape(F, C, 8, 2)
    wstk = np.ascontiguousarray(
        arr.transpose(3, 1, 2, 0).reshape(128, 512).astype(ml_dtypes.bfloat16)
    )
    bias2 = np.concatenate([b, b]).astype(np.float32).reshape(128, 1)
    bias2 = np.ascontiguousarray(bias2)
    return wstk, bias2


def kernel(x, w, b):
    import ml_dtypes

    from concourse.bass_utils import run_bass_kernel_spmd

    x = np.asarray(x, dtype=np.float32)
    w = np.asarray(w, dtype=np.float32)
    b = np.asarray(b, dtype=np.float32)
    assert x.shape == (N, C, W) and w.shape == (F, C, WW) and b.shape == (F,)

    nc = _get_nc()
    wstk, bias2 = _host_prep(w, b)
    xbf = np.ascontiguousarray(x.astype(ml_dtypes.bfloat16))
    in_maps = [
        {
            "x": np.ascontiguousarray(xbf[i * NPC : (i + 1) * NPC]),
            "wstk": wstk,
            "bias2": bias2,
        }
        for i in range(N_CORES)
    ]
    res = run_bass_kernel_spmd(nc, in_maps, core_ids=list(range(N_CORES)))
    out = np.concatenate([r["out"] for r in res.results], axis=0)
    return out.astype(np.float32)
